# revision 20
# baseline (speedup 1.0000x reference)
"""Trainium2 Bass kernel for nn_AdaptiveEmbeddingI2T (retrieval_knn).

Data-parallel over the image axis: 8 cores, 6 images each; cap_embed and
the gamma/beta projection weights are replicated. Each core computes its
(6, 48) slab of the similarity matrix; the host concatenates.

Math notes (per image i, caption c, channel d):
  txt[c,d,t] = sc[i,d] * cap[d,c,t] + bi[i,d]      (BN + FiLM folded into one affine)
  mask = softmax_t(txt);  tv[c,d] = max_t(mask*txt) = max_t(f(txt_t)) / sum_t exp(txt_t)
  f(x) = x*exp(x) is decreasing-then-increasing, so max_t f(txt_t) is attained at
  an endpoint of txt's range; txt is affine in cap, so the endpoints come from
  min_t/max_t of cap -- computed ONCE (image-independent).
  sims[i,c] = (q_sum . tv) / (||q_sum|| * ||tv||)   (the 1/R mean factor cancels)
"""

import os
import sys

import numpy as np


def _ensure_import():
    try:
        import concourse.bass  # noqa: F401
        return
    except Exception:
        pass
    for p in ("/opt/trn_rl_repo", "/root/.axon_site/_ro/trn_rl_repo"):
        if os.path.isdir(p) and p not in sys.path:
            sys.path.insert(0, p)
    import concourse.bass  # noqa: F401


_ensure_import()


def _install_axon_profile_shim():
    """The image's antenv lacks axon_hooks; synthesize it so trace=True under
    axon can register the ctypes NTFF profiling hook from trn_boot."""
    try:
        import antenv.axon_hooks  # noqa: F401
        return
    except Exception:
        pass
    try:
        import types

        import antenv

        mod = types.ModuleType("antenv.axon_hooks")
        holder = {"h": None}
        mod.set_axon_ntff_profile_hook = lambda h: holder.__setitem__("h", h)
        mod.get_axon_ntff_profile_hook = lambda: holder["h"]
        sys.modules["antenv.axon_hooks"] = mod
        antenv.axon_hooks = mod

        boot_dir = "/root/.axon_site/trn_agent_boot"
        so_path = "/opt/axon/libaxon_pjrt.so"
        if os.path.isdir(boot_dir) and os.path.exists(so_path):
            if boot_dir not in sys.path:
                sys.path.insert(0, boot_dir)
            import trn_boot

            h = trn_boot._ntff_profile_via_ctypes(so_path)
            if h is not None:
                mod.set_axon_ntff_profile_hook(h)
    except Exception:
        pass


_install_axon_profile_shim()

from contextlib import ExitStack  # noqa: E402

import ml_dtypes  # noqa: E402

import concourse.bass as bass  # noqa: E402
import concourse.bacc as bacc  # noqa: E402
import concourse.tile as tile  # noqa: E402
from concourse import mybir  # noqa: E402
from concourse.bass_utils import run_bass_kernel_spmd  # noqa: E402

F32 = mybir.dt.float32
BF16 = mybir.dt.bfloat16
AX = mybir.AxisListType
ALU = mybir.AluOpType
ACT = mybir.ActivationFunctionType

D, BI, BC, R, T = 1024, 48, 48, 36, 40
NCORES = 8
ISH = BI // NCORES  # 6 images per core
NCH = D // 128      # 8 d-chunks
EPS = 1e-5
TH = T // 2         # 20
TQ = T // 4         # 10


def build_bass():
    nc = bacc.Bacc("TRN2", target_bir_lowering=False)
    capT = nc.declare_dram_parameter("capT", [D, BC, T], BF16, isOutput=False)
    imgT = nc.declare_dram_parameter("imgT", [D, ISH, R], F32, isOutput=False)
    wgT = nc.declare_dram_parameter("wgT", [D, D], BF16, isOutput=False)
    wbT = nc.declare_dram_parameter("wbT", [D, D], BF16, isOutput=False)
    bgp = nc.declare_dram_parameter("bgp", [128, NCH], F32, isOutput=False)
    bbp = nc.declare_dram_parameter("bbp", [128, NCH], F32, isOutput=False)
    ident = nc.declare_dram_parameter("ident", [ISH, ISH], F32, isOutput=False)
    out_e = nc.declare_dram_parameter("out", [ISH, BC], F32, isOutput=True)

    with ExitStack() as ctx:
        tc = ctx.enter_context(tile.TileContext(nc))
        const = ctx.enter_context(tc.tile_pool(name="const", bufs=1))
        work = ctx.enter_context(tc.tile_pool(name="work", bufs=4))
        ipool = ctx.enter_context(tc.tile_pool(name="ipool", bufs=2))
        srpool = ctx.enter_context(tc.tile_pool(name="srpool", bufs=8))
        small = ctx.enter_context(tc.tile_pool(name="small", bufs=4))
        ppool = ctx.enter_context(tc.tile_pool(name="psum_main", bufs=2, space="PSUM"))
        ppre = ctx.enter_context(tc.tile_pool(name="psum_prep", bufs=1, space="PSUM"))

        # ---------------- persistent tiles ----------------
        cap_sb = const.tile([128, NCH, BC, T], BF16)
        img_sb = const.tile([128, NCH, ISH, R], F32)
        wg_sb = const.tile([128, NCH, D], BF16)
        wb_sb = const.tile([128, NCH, D], BF16)
        bg_sb = const.tile([128, NCH], F32)
        bb_sb = const.tile([128, NCH], F32)
        bg1_sb = const.tile([128, NCH], F32)
        ones_sb = const.tile([128, 1], F32)
        eps_sb = const.tile([128, 1], F32)
        id6 = const.tile([ISH, ISH], F32)
        xmm = const.tile([128, NCH, BC, 2], F32)
        q_sb = const.tile([128, NCH, ISH], F32)
        q_bf = const.tile([128, NCH, ISH], BF16)
        mv_all = const.tile([128, NCH, 2], F32)
        lnv = const.tile([128, NCH], F32)
        istd = const.tile([128, NCH], F32)
        a_sb = const.tile([128, NCH, ISH], F32)
        b_sb = const.tile([128, NCH, ISH], F32)
        sc_sb = const.tile([128, NCH, ISH], F32)
        bi_sb = const.tile([128, NCH, ISH], F32)
        tmp_sb = const.tile([128, NCH, ISH], F32)
        gfl = const.tile([ISH, D], F32)
        bfl = const.tile([ISH, D], F32)
        nq_s = const.tile([1, ISH], F32)
        rq_sb = const.tile([1, ISH], F32)
        dot_all = const.tile([1, ISH, BC], F32)
        ss_all = const.tile([1, ISH, BC], F32)
        lnss = const.tile([1, ISH, BC], F32)
        rsd = const.tile([1, ISH, BC], F32)
        prod_all = const.tile([1, ISH, BC], F32)
        out_sb = const.tile([1, ISH, BC], F32)

        capT_v = capT[:].rearrange("(k p) c t -> p k c t", p=128)
        imgT_v = imgT[:].rearrange("(k p) i r -> p k i r", p=128)
        wgT_v = wgT[:].rearrange("(k p) d -> p k d", p=128)
        wbT_v = wbT[:].rearrange("(k p) d -> p k d", p=128)

        nc.vector.memset(ones_sb[:], 1.0)
        nc.vector.memset(eps_sb[:], EPS)

        # ---------------- DMA in ----------------
        # HWDGE via the sync engine; Bacc's event-semaphore legalization
        # splits any multi-domain waits on consumers. Order by need: img/W
        # feed the gamma projections (longest prep chain), cap streams after.
        nc.sync.dma_start(out=img_sb[:], in_=imgT_v[:])
        nc.sync.dma_start(out=id6[:], in_=ident[:])
        nc.sync.dma_start(out=bg_sb[:], in_=bgp[:])
        nc.sync.dma_start(out=bb_sb[:], in_=bbp[:])
        for k in range(NCH):
            nc.sync.dma_start(out=wg_sb[:, k], in_=wgT_v[:, k])
            nc.sync.dma_start(out=wb_sb[:, k], in_=wbT_v[:, k])
        for k in range(NCH):
            nc.sync.dma_start(out=cap_sb[:, k], in_=capT_v[:, k])

        # ---------------- prep ----------------
        nc.vector.tensor_scalar_add(bg1_sb[:], bg_sb[:], 1.0)
        bb1_sb = const.tile([128, NCH], F32)
        nc.vector.tensor_copy(bb1_sb[:], bb_sb[:])

        for k in range(NCH):
            # image query: sum over regions (mean folded later)
            nc.vector.reduce_sum(q_sb[:, k], img_sb[:, k], axis=AX.X)
            nc.vector.tensor_copy(q_bf[:, k], q_sb[:, k])

        # gamma/beta projections: psum (ISH, 512) accumulated over k-chunks
        for w_sb, flat in ((wg_sb, gfl), (wb_sb, bfl)):
            for h in range(2):
                ps = ppre.tile([ISH, 512], F32, tag="gb")
                for k in range(NCH):
                    nc.tensor.matmul(
                        ps[:], q_bf[:, k], w_sb[:, k, h * 512:(h + 1) * 512],
                        start=(k == 0), stop=(k == NCH - 1),
                    )
                nc.scalar.copy(flat[:, h * 512:(h + 1) * 512], ps[:])

        # per-chunk: BN stats -> istd -> fold, plus min/max trees; everything
        # chunk k needs is ready as soon as cap chunk k lands, so the main
        # loop's exp stream can start long before prep of later chunks ends.
        for k in range(NCH):
            cap_flat = cap_sb[:, k].rearrange("p c t -> p (c t)")
            stats = small.tile([128, 4, 6], F32, tag="bnstats")
            for g in range(4):
                nc.vector.bn_stats(out=stats[:, g], in_=cap_flat[:, g * 480:(g + 1) * 480])
            nc.vector.bn_aggr(out=mv_all[:, k], in_=stats[:])
            nc.scalar.activation(lnv[:, k:k + 1], mv_all[:, k, 1:2], func=ACT.Ln,
                                 bias=eps_sb[:], scale=1.0)
            nc.scalar.activation(istd[:, k:k + 1], lnv[:, k:k + 1], func=ACT.Exp, scale=-0.5)

            ptg = ppre.tile([128, ISH], F32, tag="tp")
            nc.tensor.transpose(ptg[:], gfl[:, k * 128:(k + 1) * 128], id6[:])
            nc.scalar.activation(a_sb[:, k], ptg[:], func=ACT.Identity,
                                 scale=1.0 / R, bias=bg1_sb[:, k:k + 1])
            ptb = ppre.tile([128, ISH], F32, tag="tp")
            nc.tensor.transpose(ptb[:], bfl[:, k * 128:(k + 1) * 128], id6[:])
            nc.scalar.activation(b_sb[:, k], ptb[:], func=ACT.Identity,
                                 scale=1.0 / R, bias=bb1_sb[:, k:k + 1])
            # sc = a * istd ; bi = b - sc * mean
            nc.vector.tensor_scalar(sc_sb[:, k], a_sb[:, k], istd[:, k:k + 1], None, op0=ALU.mult)
            nc.vector.tensor_scalar(tmp_sb[:, k], sc_sb[:, k], mv_all[:, k, 0:1], None, op0=ALU.mult)
            nc.vector.tensor_sub(bi_sb[:, k], b_sb[:, k], tmp_sb[:, k])

            # min/max over t per (c, d): widest tree stage on the idle GpSimd
            xt1 = small.tile([128, BC, TH], BF16, tag="xt1")
            xt2 = small.tile([128, BC, TQ], BF16, tag="xt2")
            nc.vector.tensor_tensor(xt1[:], cap_sb[:, k, :, 0:TH], cap_sb[:, k, :, TH:T], op=ALU.max)
            nc.vector.tensor_tensor(xt2[:], xt1[:, :, 0:TQ], xt1[:, :, TQ:TH], op=ALU.max)
            nc.vector.tensor_reduce(xmm[:, k, :, 1:2], xt2[:], axis=AX.X, op=ALU.max)
            xn1 = small.tile([128, BC, TH], BF16, tag="xn1")
            xn2 = small.tile([128, BC, TQ], BF16, tag="xn2")
            nc.vector.tensor_tensor(xn1[:], cap_sb[:, k, :, 0:TH], cap_sb[:, k, :, TH:T], op=ALU.min)
            nc.vector.tensor_tensor(xn2[:], xn1[:, :, 0:TQ], xn1[:, :, TQ:TH], op=ALU.min)
            nc.vector.tensor_reduce(xmm[:, k, :, 0:1], xn2[:], axis=AX.X, op=ALU.min)

        # ||q_sum||^-1 per image, landing on partition 0 as (1, ISH)
        nqp = ppre.tile([1, ISH], F32, tag="nq")
        sq_all = const.tile([128, NCH, ISH], F32)
        nc.vector.tensor_mul(sq_all[:], q_sb[:], q_sb[:])
        for k in range(NCH):
            nc.tensor.matmul(nqp[:], ones_sb[:], sq_all[:, k], start=(k == 0), stop=(k == NCH - 1))
        nc.scalar.activation(nq_s[:], nqp[:], func=ACT.Ln, bias=0.0, scale=1.0)
        nc.scalar.activation(rq_sb[:], nq_s[:], func=ACT.Exp, scale=-0.5)

        # ---------------- main loop ----------------
        for i in range(ISH):
            c12 = ipool.tile([128, NCH, BC, 2], F32, tag="c12")
            e12 = ipool.tile([128, NCH, BC, 2], F32, tag="e12")
            f12 = ipool.tile([128, NCH, BC, 2], F32, tag="f12")
            m_all = ipool.tile([128, NCH, BC], F32, tag="m")
            for k in range(NCH):
                nc.vector.tensor_scalar(
                    c12[:, k], xmm[:, k], sc_sb[:, k, i:i + 1], bi_sb[:, k, i:i + 1],
                    op0=ALU.mult, op1=ALU.add,
                )
            nc.scalar.activation(e12[:], c12[:], func=ACT.Exp)
            nc.vector.tensor_mul(f12[:], c12[:], e12[:])
            nc.vector.tensor_reduce(m_all[:], f12[:], axis=AX.X, op=ALU.max)

            pd = ppool.tile([1, 2 * BC], F32, tag="pdot")
            pss = ppool.tile([1, 2 * BC], F32, tag="pss")
            for k in range(NCH):
                y = work.tile([128, BC, T], BF16, tag="y")
                nc.scalar.activation(y[:], cap_sb[:, k], func=ACT.Exp,
                                     scale=sc_sb[:, k, i:i + 1], bias=bi_sb[:, k, i:i + 1])
                ys1 = work.tile([128, BC, TH], BF16, tag="ys1")
                # split the widest sum stage between DVE and the idle GpSimd
                eng1 = nc.gpsimd if (k % 2 == 0) else nc.vector
                eng1.tensor_add(ys1[:], y[:, :, 0:TH], y[:, :, TH:T])
                ys2 = work.tile([128, BC, TQ], BF16, tag="ys2")
                nc.vector.tensor_add(ys2[:], ys1[:, :, 0:TQ], ys1[:, :, TQ:TH])
                s_t = srpool.tile([128, BC], F32, tag="s")
                nc.vector.reduce_sum(s_t[:], ys2[:], axis=AX.X)
                r_t = srpool.tile([128, BC], F32, tag="r")
                nc.vector.reciprocal_approx_fast(r_t[:], s_t[:])
                tv = work.tile([128, 2, BC], F32, tag="tv")
                nc.vector.tensor_mul(tv[:, 0], m_all[:, k], r_t[:])
                nc.vector.tensor_mul(tv[:, 1], tv[:, 0], tv[:, 0])
                nc.tensor.matmul(pd[:], q_sb[:, k, i:i + 1], tv[:],
                                 start=(k == 0), stop=(k == NCH - 1))
                nc.tensor.matmul(pss[:], ones_sb[:], tv[:],
                                 start=(k == 0), stop=(k == NCH - 1))
            nc.vector.tensor_copy(dot_all[:, i], pd[0:1, 0:BC])
            nc.vector.tensor_copy(ss_all[:, i], pss[0:1, BC:2 * BC])

        # ---------------- tail ----------------
        nc.scalar.activation(lnss[:], ss_all[:], func=ACT.Ln, bias=0.0, scale=1.0)
        nc.scalar.activation(rsd[:], lnss[:], func=ACT.Exp, scale=-0.5)
        nc.vector.tensor_mul(prod_all[:], dot_all[:], rsd[:])
        for i in range(ISH):
            nc.vector.tensor_scalar(out_sb[0:1, i], prod_all[0:1, i],
                                    rq_sb[0:1, i:i + 1], None, op0=ALU.mult)
        nc.sync.dma_start(out=out_e[:].rearrange("i c -> (i c)"),
                          in_=out_sb[:].rearrange("p i c -> p (i c)"))

    nc.compile()
    return nc


_NC = None
LAST_RESULT = None


def _get_nc():
    global _NC
    if _NC is None:
        _NC = build_bass()
    return _NC


def kernel(img_embed, cap_embed, lens=None, W_gamma=None, b_gamma=None,
           W_beta=None, b_beta=None, **_unused):
    global LAST_RESULT
    img = np.asarray(img_embed, dtype=np.float32)
    cap = np.asarray(cap_embed, dtype=np.float32)
    Wg = np.asarray(W_gamma, dtype=np.float32)
    Wb = np.asarray(W_beta, dtype=np.float32)
    bg = np.asarray(b_gamma, dtype=np.float32)
    bb = np.asarray(b_beta, dtype=np.float32)

    capT = np.ascontiguousarray(cap.transpose(2, 0, 1)).astype(ml_dtypes.bfloat16)
    wgT = np.ascontiguousarray(Wg.T).astype(ml_dtypes.bfloat16)
    wbT = np.ascontiguousarray(Wb.T).astype(ml_dtypes.bfloat16)
    bgp = np.ascontiguousarray(bg.reshape(NCH, 128).T).astype(np.float32)
    bbp = np.ascontiguousarray(bb.reshape(NCH, 128).T).astype(np.float32)

    in_maps = []
    for c in range(NCORES):
        i0 = c * ISH
        imgT_c = np.ascontiguousarray(
            img[i0:i0 + ISH].transpose(2, 0, 1)).astype(np.float32)
        in_maps.append({
            "capT": capT, "imgT": imgT_c, "wgT": wgT, "wbT": wbT,
            "bgp": bgp, "bbp": bbp, "ident": np.eye(ISH, dtype=np.float32),
        })

    nc = _get_nc()
    res = run_bass_kernel_spmd(nc, in_maps, core_ids=list(range(NCORES)))
    LAST_RESULT = res
    out = np.concatenate(
        [np.asarray(res.results[c]["out"], dtype=np.float32) for c in range(NCORES)],
        axis=0,
    )
    return out


if __name__ == "__main__":
    rng = np.random.default_rng(0)
    ins = dict(
        img_embed=rng.standard_normal((BI, R, D), dtype=np.float32),
        cap_embed=rng.standard_normal((BC, T, D), dtype=np.float32),
        lens=rng.integers(1, T, size=(BC,)),
        W_gamma=(rng.standard_normal((D, D), dtype=np.float32) / np.sqrt(D)).astype(np.float32),
        b_gamma=np.zeros((D,), np.float32),
        W_beta=(rng.standard_normal((D, D), dtype=np.float32) / np.sqrt(D)).astype(np.float32),
        b_beta=np.zeros((D,), np.float32),
    )
    o = kernel(**ins)
    print(o.shape, o.dtype, float(np.abs(o).mean()))


# revision 25
# speedup vs baseline: 1.1861x; 1.1861x over previous
"""Trainium2 Bass kernel for nn_AdaptiveEmbeddingI2T (retrieval_knn).

Data-parallel over the image axis: 8 cores, 6 images each; cap_embed and
the gamma/beta projection weights are replicated. Each core computes its
(6, 48) slab of the similarity matrix; the host concatenates.

Math notes (per image i, caption c, channel d):
  txt[c,d,t] = sc[i,d] * cap[d,c,t] + bi[i,d]      (BN + FiLM folded into one affine)
  mask = softmax_t(txt);  tv[c,d] = max_t(mask*txt) = max_t(f(txt_t)) / sum_t exp(txt_t)
  f(x) = x*exp(x) is decreasing-then-increasing, so max_t f(txt_t) is attained at
  an endpoint of txt's range; txt is affine in cap, so the endpoints come from
  min_t/max_t of cap -- computed ONCE (image-independent).
  sims[i,c] = (q_sum . tv) / (||q_sum|| * ||tv||)   (the 1/R mean factor cancels)
"""

import os
import sys

import numpy as np


def _ensure_import():
    try:
        import concourse.bass  # noqa: F401
        return
    except Exception:
        pass
    for p in ("/opt/trn_rl_repo", "/root/.axon_site/_ro/trn_rl_repo"):
        if os.path.isdir(p) and p not in sys.path:
            sys.path.insert(0, p)
    import concourse.bass  # noqa: F401


_ensure_import()


def _install_axon_profile_shim():
    """The image's antenv lacks axon_hooks; synthesize it so trace=True under
    axon can register the ctypes NTFF profiling hook from trn_boot."""
    try:
        import antenv.axon_hooks  # noqa: F401
        return
    except Exception:
        pass
    try:
        import types

        import antenv

        mod = types.ModuleType("antenv.axon_hooks")
        holder = {"h": None}
        mod.set_axon_ntff_profile_hook = lambda h: holder.__setitem__("h", h)
        mod.get_axon_ntff_profile_hook = lambda: holder["h"]
        sys.modules["antenv.axon_hooks"] = mod
        antenv.axon_hooks = mod

        boot_dir = "/root/.axon_site/trn_agent_boot"
        so_path = "/opt/axon/libaxon_pjrt.so"
        if os.path.isdir(boot_dir) and os.path.exists(so_path):
            if boot_dir not in sys.path:
                sys.path.insert(0, boot_dir)
            import trn_boot

            h = trn_boot._ntff_profile_via_ctypes(so_path)
            if h is not None:
                mod.set_axon_ntff_profile_hook(h)
    except Exception:
        pass


_install_axon_profile_shim()

from contextlib import ExitStack  # noqa: E402

import ml_dtypes  # noqa: E402

import concourse.bass as bass  # noqa: E402
import concourse.bacc as bacc  # noqa: E402
import concourse.tile as tile  # noqa: E402
from concourse import mybir  # noqa: E402
from concourse.bass_utils import run_bass_kernel_spmd  # noqa: E402

F32 = mybir.dt.float32
BF16 = mybir.dt.bfloat16
AX = mybir.AxisListType
ALU = mybir.AluOpType
ACT = mybir.ActivationFunctionType

D, BI, BC, R, T = 1024, 48, 48, 36, 40
NCORES = 8
ISH = BI // NCORES  # 6 images per core
NCH = D // 128      # 8 d-chunks
EPS = 1e-5
TH = T // 2         # 20
TQ = T // 4         # 10


def build_bass():
    nc = bacc.Bacc("TRN2", target_bir_lowering=False)
    capT = nc.declare_dram_parameter("capT", [D, BC, T], BF16, isOutput=False)
    imgT = nc.declare_dram_parameter("imgT", [D, ISH, R], F32, isOutput=False)
    wgT = nc.declare_dram_parameter("wgT", [D, D], BF16, isOutput=False)
    wbT = nc.declare_dram_parameter("wbT", [D, D], BF16, isOutput=False)
    bgp = nc.declare_dram_parameter("bgp", [128, NCH], F32, isOutput=False)
    bbp = nc.declare_dram_parameter("bbp", [128, NCH], F32, isOutput=False)
    ident = nc.declare_dram_parameter("ident", [ISH, ISH], F32, isOutput=False)
    out_e = nc.declare_dram_parameter("out", [ISH, BC], F32, isOutput=True)

    with ExitStack() as ctx:
        tc = ctx.enter_context(tile.TileContext(nc))
        const = ctx.enter_context(tc.tile_pool(name="const", bufs=1))
        work = ctx.enter_context(tc.tile_pool(name="work", bufs=4))
        ipool = ctx.enter_context(tc.tile_pool(name="ipool", bufs=2))
        srpool = ctx.enter_context(tc.tile_pool(name="srpool", bufs=16))
        small = ctx.enter_context(tc.tile_pool(name="small", bufs=4))
        ppool = ctx.enter_context(tc.tile_pool(name="psum_main", bufs=2, space="PSUM"))
        ppre = ctx.enter_context(tc.tile_pool(name="psum_prep", bufs=1, space="PSUM"))

        # ---------------- persistent tiles ----------------
        cap_sb = const.tile([128, NCH, BC, T], BF16)
        img_sb = const.tile([128, NCH, ISH, R], F32)
        wg_sb = const.tile([128, NCH, D], BF16)
        wb_sb = const.tile([128, NCH, D], BF16)
        bg_sb = const.tile([128, NCH], F32)
        bb_sb = const.tile([128, NCH], F32)
        bg1_sb = const.tile([128, NCH], F32)
        ones_sb = const.tile([128, 1], F32)
        eps_sb = const.tile([128, 1], F32)
        id6 = const.tile([ISH, ISH], F32)
        xmm = const.tile([128, NCH, BC, 2], F32)
        q_sb = const.tile([128, NCH, ISH], F32)
        q_bf = const.tile([128, NCH, ISH], BF16)
        mv_all = const.tile([128, NCH, 2], F32)
        lnv = const.tile([128, NCH], F32)
        istd = const.tile([128, NCH], F32)
        a_sb = const.tile([128, NCH, ISH], F32)
        b_sb = const.tile([128, NCH, ISH], F32)
        sc_sb = const.tile([128, NCH, ISH], F32)
        bi_sb = const.tile([128, NCH, ISH], F32)
        tmp_sb = const.tile([128, NCH, ISH], F32)
        gfl = const.tile([ISH, D], F32)
        bfl = const.tile([ISH, D], F32)
        nq_s = const.tile([1, ISH], F32)
        rq_sb = const.tile([1, ISH], F32)
        dot_all = const.tile([1, ISH, BC], F32)
        ss_all = const.tile([1, ISH, BC], F32)
        lnss = const.tile([1, ISH, BC], F32)
        rsd = const.tile([1, ISH, BC], F32)
        prod_all = const.tile([1, ISH, BC], F32)
        out_sb = const.tile([1, ISH, BC], F32)

        capT_v = capT[:].rearrange("(k p) c t -> p k c t", p=128)
        imgT_v = imgT[:].rearrange("(k p) i r -> p k i r", p=128)
        wgT_v = wgT[:].rearrange("(k p) d -> p k d", p=128)
        wbT_v = wbT[:].rearrange("(k p) d -> p k d", p=128)

        nc.vector.memset(ones_sb[:], 1.0)
        nc.vector.memset(eps_sb[:], EPS)

        # ---------------- DMA in ----------------
        # HWDGE via the sync engine; Bacc's event-semaphore legalization
        # splits any multi-domain waits on consumers. Order by need: img/W
        # feed the gamma projections (longest prep chain), cap streams after.
        nc.sync.dma_start(out=img_sb[:], in_=imgT_v[:])
        nc.sync.dma_start(out=id6[:], in_=ident[:])
        nc.sync.dma_start(out=bg_sb[:], in_=bgp[:])
        nc.sync.dma_start(out=bb_sb[:], in_=bbp[:])
        for k in range(NCH):
            nc.sync.dma_start(out=cap_sb[:, k], in_=capT_v[:, k])
        for k in range(NCH):
            nc.sync.dma_start(out=wg_sb[:, k], in_=wgT_v[:, k])
            nc.sync.dma_start(out=wb_sb[:, k], in_=wbT_v[:, k])

        # ---------------- prep ----------------
        nc.vector.tensor_scalar_add(bg1_sb[:], bg_sb[:], 1.0)
        bb1_sb = const.tile([128, NCH], F32)
        nc.vector.tensor_copy(bb1_sb[:], bb_sb[:])

        for k in range(NCH):
            # image query: sum over regions (mean folded later)
            nc.vector.reduce_sum(q_sb[:, k], img_sb[:, k], axis=AX.X)
            nc.vector.tensor_copy(q_bf[:, k], q_sb[:, k])

        # gamma/beta projections: psum (ISH, 512) accumulated over k-chunks
        for w_sb, flat in ((wg_sb, gfl), (wb_sb, bfl)):
            for h in range(2):
                ps = ppre.tile([ISH, 512], F32, tag="gb")
                for k in range(NCH):
                    nc.tensor.matmul(
                        ps[:], q_bf[:, k], w_sb[:, k, h * 512:(h + 1) * 512],
                        start=(k == 0), stop=(k == NCH - 1),
                    )
                nc.scalar.copy(flat[:, h * 512:(h + 1) * 512], ps[:])

        # per-chunk BN -> istd -> fold first: sc/bi[k] is all the y-exp stream
        # needs, so ACT unblocks as early as possible.
        for k in range(NCH):
            cap_flat = cap_sb[:, k].rearrange("p c t -> p (c t)")
            stats = small.tile([128, 4, 6], F32, tag="bnstats")
            for g in range(4):
                nc.vector.bn_stats(out=stats[:, g], in_=cap_flat[:, g * 480:(g + 1) * 480])
            nc.vector.bn_aggr(out=mv_all[:, k], in_=stats[:])
            nc.scalar.activation(lnv[:, k:k + 1], mv_all[:, k, 1:2], func=ACT.Ln,
                                 bias=eps_sb[:], scale=1.0)
            nc.scalar.activation(istd[:, k:k + 1], lnv[:, k:k + 1], func=ACT.Exp, scale=-0.5)

            ptg = ppre.tile([128, ISH], F32, tag="tp")
            nc.tensor.transpose(ptg[:], gfl[:, k * 128:(k + 1) * 128], id6[:])
            nc.scalar.activation(a_sb[:, k], ptg[:], func=ACT.Identity,
                                 scale=1.0 / R, bias=bg1_sb[:, k:k + 1])
            ptb = ppre.tile([128, ISH], F32, tag="tp")
            nc.tensor.transpose(ptb[:], bfl[:, k * 128:(k + 1) * 128], id6[:])
            nc.scalar.activation(b_sb[:, k], ptb[:], func=ACT.Identity,
                                 scale=1.0 / R, bias=bb1_sb[:, k:k + 1])
            # sc = a * istd ; bi = b - sc * mean
            nc.vector.tensor_scalar(sc_sb[:, k], a_sb[:, k], istd[:, k:k + 1], None, op0=ALU.mult)
            nc.vector.tensor_scalar(tmp_sb[:, k], sc_sb[:, k], mv_all[:, k, 0:1], None, op0=ALU.mult)
            nc.vector.tensor_sub(bi_sb[:, k], b_sb[:, k], tmp_sb[:, k])

        # min/max over t per (c, d): 2x bf16 tree stages + final 1x reduce.
        # Off the y-exp critical path; only gates the first tv-multiplies.
        for k in range(NCH):
            xt1 = small.tile([128, BC, TH], BF16, tag="xt1")
            xt2 = small.tile([128, BC, TQ], BF16, tag="xt2")
            nc.vector.tensor_tensor(xt1[:], cap_sb[:, k, :, 0:TH], cap_sb[:, k, :, TH:T], op=ALU.max)
            nc.vector.tensor_tensor(xt2[:], xt1[:, :, 0:TQ], xt1[:, :, TQ:TH], op=ALU.max)
            nc.vector.tensor_reduce(xmm[:, k, :, 1:2], xt2[:], axis=AX.X, op=ALU.max)
            xn1 = small.tile([128, BC, TH], BF16, tag="xn1")
            xn2 = small.tile([128, BC, TQ], BF16, tag="xn2")
            nc.vector.tensor_tensor(xn1[:], cap_sb[:, k, :, 0:TH], cap_sb[:, k, :, TH:T], op=ALU.min)
            nc.vector.tensor_tensor(xn2[:], xn1[:, :, 0:TQ], xn1[:, :, TQ:TH], op=ALU.min)
            nc.vector.tensor_reduce(xmm[:, k, :, 0:1], xn2[:], axis=AX.X, op=ALU.min)

        # ||q_sum||^-1 per image, landing on partition 0 as (1, ISH)
        nqp = ppre.tile([1, ISH], F32, tag="nq")
        sq_all = const.tile([128, NCH, ISH], F32)
        nc.vector.tensor_mul(sq_all[:], q_sb[:], q_sb[:])
        for k in range(NCH):
            nc.tensor.matmul(nqp[:], ones_sb[:], sq_all[:, k], start=(k == 0), stop=(k == NCH - 1))
        nc.scalar.activation(nq_s[:], nqp[:], func=ACT.Ln, bias=0.0, scale=1.0)
        nc.scalar.activation(rq_sb[:], nq_s[:], func=ACT.Exp, scale=-0.5)

        # ---------------- main loop ----------------
        for i in range(ISH):
            c12 = ipool.tile([128, NCH, BC, 2], F32, tag="c12")
            e12 = ipool.tile([128, NCH, BC, 2], F32, tag="e12")
            f12 = ipool.tile([128, NCH, BC, 2], F32, tag="f12")
            m_all = ipool.tile([128, NCH, BC], F32, tag="m")
            for k in range(NCH):
                nc.vector.tensor_scalar(
                    c12[:, k], xmm[:, k], sc_sb[:, k, i:i + 1], bi_sb[:, k, i:i + 1],
                    op0=ALU.mult, op1=ALU.add,
                )
            nc.scalar.activation(e12[:], c12[:], func=ACT.Exp)
            nc.vector.tensor_mul(f12[:], c12[:], e12[:])
            nc.vector.tensor_tensor(m_all[:], f12[:, :, :, 0], f12[:, :, :, 1], op=ALU.max)

            pd = ppool.tile([1, 2 * BC], F32, tag="pdot")
            pss = ppool.tile([1, 2 * BC], F32, tag="pss")
            for k in range(NCH):
                y = work.tile([128, BC, T], BF16, tag="y")
                nc.scalar.activation(y[:], cap_sb[:, k], func=ACT.Exp,
                                     scale=sc_sb[:, k, i:i + 1], bias=bi_sb[:, k, i:i + 1])
                ys1 = work.tile([128, BC, TH], BF16, tag="ys1")
                nc.vector.tensor_add(ys1[:], y[:, :, 0:TH], y[:, :, TH:T])
                ys2 = work.tile([128, BC, TQ], BF16, tag="ys2")
                nc.vector.tensor_add(ys2[:], ys1[:, :, 0:TQ], ys1[:, :, TQ:TH])
                s_t = srpool.tile([128, BC], F32, tag="s")
                nc.vector.reduce_sum(s_t[:], ys2[:], axis=AX.X)
                r_t = srpool.tile([128, BC], F32, tag="r")
                nc.vector.reciprocal_approx_fast(r_t[:], s_t[:])
                tv = work.tile([128, 2, BC], F32, tag="tv")
                nc.vector.tensor_mul(tv[:, 0], m_all[:, k], r_t[:])
                nc.vector.tensor_mul(tv[:, 1], tv[:, 0], tv[:, 0])
                nc.tensor.matmul(pd[:], q_sb[:, k, i:i + 1], tv[:],
                                 start=(k == 0), stop=(k == NCH - 1))
                nc.tensor.matmul(pss[:], ones_sb[:], tv[:],
                                 start=(k == 0), stop=(k == NCH - 1))
            nc.vector.tensor_copy(dot_all[:, i], pd[0:1, 0:BC])
            nc.vector.tensor_copy(ss_all[:, i], pss[0:1, BC:2 * BC])

        # ---------------- tail ----------------
        nc.scalar.activation(lnss[:], ss_all[:], func=ACT.Ln, bias=0.0, scale=1.0)
        nc.scalar.activation(rsd[:], lnss[:], func=ACT.Exp, scale=-0.5)
        nc.vector.tensor_mul(prod_all[:], dot_all[:], rsd[:])
        for i in range(ISH):
            nc.vector.tensor_scalar(out_sb[0:1, i], prod_all[0:1, i],
                                    rq_sb[0:1, i:i + 1], None, op0=ALU.mult)
        nc.sync.dma_start(out=out_e[:].rearrange("i c -> (i c)"),
                          in_=out_sb[:].rearrange("p i c -> p (i c)"))

    nc.compile()
    return nc


_NC = None
LAST_RESULT = None


def _get_nc():
    global _NC
    if _NC is None:
        _NC = build_bass()
    return _NC


def kernel(img_embed, cap_embed, lens=None, W_gamma=None, b_gamma=None,
           W_beta=None, b_beta=None, **_unused):
    global LAST_RESULT
    img = np.asarray(img_embed, dtype=np.float32)
    cap = np.asarray(cap_embed, dtype=np.float32)
    Wg = np.asarray(W_gamma, dtype=np.float32)
    Wb = np.asarray(W_beta, dtype=np.float32)
    bg = np.asarray(b_gamma, dtype=np.float32)
    bb = np.asarray(b_beta, dtype=np.float32)

    capT = np.ascontiguousarray(cap.transpose(2, 0, 1)).astype(ml_dtypes.bfloat16)
    wgT = np.ascontiguousarray(Wg.T).astype(ml_dtypes.bfloat16)
    wbT = np.ascontiguousarray(Wb.T).astype(ml_dtypes.bfloat16)
    bgp = np.ascontiguousarray(bg.reshape(NCH, 128).T).astype(np.float32)
    bbp = np.ascontiguousarray(bb.reshape(NCH, 128).T).astype(np.float32)

    in_maps = []
    for c in range(NCORES):
        i0 = c * ISH
        imgT_c = np.ascontiguousarray(
            img[i0:i0 + ISH].transpose(2, 0, 1)).astype(np.float32)
        in_maps.append({
            "capT": capT, "imgT": imgT_c, "wgT": wgT, "wbT": wbT,
            "bgp": bgp, "bbp": bbp, "ident": np.eye(ISH, dtype=np.float32),
        })

    nc = _get_nc()
    res = run_bass_kernel_spmd(nc, in_maps, core_ids=list(range(NCORES)))
    LAST_RESULT = res
    out = np.concatenate(
        [np.asarray(res.results[c]["out"], dtype=np.float32) for c in range(NCORES)],
        axis=0,
    )
    return out


if __name__ == "__main__":
    rng = np.random.default_rng(0)
    ins = dict(
        img_embed=rng.standard_normal((BI, R, D), dtype=np.float32),
        cap_embed=rng.standard_normal((BC, T, D), dtype=np.float32),
        lens=rng.integers(1, T, size=(BC,)),
        W_gamma=(rng.standard_normal((D, D), dtype=np.float32) / np.sqrt(D)).astype(np.float32),
        b_gamma=np.zeros((D,), np.float32),
        W_beta=(rng.standard_normal((D, D), dtype=np.float32) / np.sqrt(D)).astype(np.float32),
        b_beta=np.zeros((D,), np.float32),
    )
    o = kernel(**ins)
    print(o.shape, o.dtype, float(np.abs(o).mean()))


# revision 26
# speedup vs baseline: 1.2651x; 1.0666x over previous
"""Trainium2 Bass kernel for nn_AdaptiveEmbeddingI2T (retrieval_knn).

Data-parallel over the image axis: 8 cores, 6 images each; cap_embed and
the gamma/beta projection weights are replicated. Each core computes its
(6, 48) slab of the similarity matrix; the host concatenates.

Math notes (per image i, caption c, channel d):
  txt[c,d,t] = sc[i,d] * cap[d,c,t] + bi[i,d]      (BN + FiLM folded into one affine)
  mask = softmax_t(txt);  tv[c,d] = max_t(mask*txt) = max_t(f(txt_t)) / sum_t exp(txt_t)
  f(x) = x*exp(x) is decreasing-then-increasing, so max_t f(txt_t) is attained at
  an endpoint of txt's range; txt is affine in cap, so the endpoints come from
  min_t/max_t of cap -- computed ONCE (image-independent).
  sims[i,c] = (q_sum . tv) / (||q_sum|| * ||tv||)   (the 1/R mean factor cancels)
"""

import os
import sys

import numpy as np


def _ensure_import():
    try:
        import concourse.bass  # noqa: F401
        return
    except Exception:
        pass
    for p in ("/opt/trn_rl_repo", "/root/.axon_site/_ro/trn_rl_repo"):
        if os.path.isdir(p) and p not in sys.path:
            sys.path.insert(0, p)
    import concourse.bass  # noqa: F401


_ensure_import()


def _install_axon_profile_shim():
    """The image's antenv lacks axon_hooks; synthesize it so trace=True under
    axon can register the ctypes NTFF profiling hook from trn_boot."""
    try:
        import antenv.axon_hooks  # noqa: F401
        return
    except Exception:
        pass
    try:
        import types

        import antenv

        mod = types.ModuleType("antenv.axon_hooks")
        holder = {"h": None}
        mod.set_axon_ntff_profile_hook = lambda h: holder.__setitem__("h", h)
        mod.get_axon_ntff_profile_hook = lambda: holder["h"]
        sys.modules["antenv.axon_hooks"] = mod
        antenv.axon_hooks = mod

        boot_dir = "/root/.axon_site/trn_agent_boot"
        so_path = "/opt/axon/libaxon_pjrt.so"
        if os.path.isdir(boot_dir) and os.path.exists(so_path):
            if boot_dir not in sys.path:
                sys.path.insert(0, boot_dir)
            import trn_boot

            h = trn_boot._ntff_profile_via_ctypes(so_path)
            if h is not None:
                mod.set_axon_ntff_profile_hook(h)
    except Exception:
        pass


_install_axon_profile_shim()

from contextlib import ExitStack  # noqa: E402

import ml_dtypes  # noqa: E402

import concourse.bass as bass  # noqa: E402
import concourse.bacc as bacc  # noqa: E402
import concourse.tile as tile  # noqa: E402
from concourse import mybir  # noqa: E402
from concourse.bass_utils import run_bass_kernel_spmd  # noqa: E402

F32 = mybir.dt.float32
BF16 = mybir.dt.bfloat16
AX = mybir.AxisListType
ALU = mybir.AluOpType
ACT = mybir.ActivationFunctionType

D, BI, BC, R, T = 1024, 48, 48, 36, 40
NCORES = 8
ISH = BI // NCORES  # 6 images per core
NCH = D // 128      # 8 d-chunks
EPS = 1e-5
TH = T // 2         # 20
TQ = T // 4         # 10


def build_bass():
    nc = bacc.Bacc("TRN2", target_bir_lowering=False)
    capT = nc.declare_dram_parameter("capT", [D, BC, T], BF16, isOutput=False)
    imgT = nc.declare_dram_parameter("imgT", [D, ISH, R], F32, isOutput=False)
    wgT = nc.declare_dram_parameter("wgT", [D, D], BF16, isOutput=False)
    wbT = nc.declare_dram_parameter("wbT", [D, D], BF16, isOutput=False)
    bgp = nc.declare_dram_parameter("bgp", [128, NCH], F32, isOutput=False)
    bbp = nc.declare_dram_parameter("bbp", [128, NCH], F32, isOutput=False)
    ident = nc.declare_dram_parameter("ident", [ISH, ISH], F32, isOutput=False)
    out_e = nc.declare_dram_parameter("out", [ISH, BC], F32, isOutput=True)

    with ExitStack() as ctx:
        tc = ctx.enter_context(tile.TileContext(nc))
        const = ctx.enter_context(tc.tile_pool(name="const", bufs=1))
        work = ctx.enter_context(tc.tile_pool(name="work", bufs=4))
        ypool = ctx.enter_context(tc.tile_pool(name="ypool", bufs=8))
        ipool = ctx.enter_context(tc.tile_pool(name="ipool", bufs=2))
        srpool = ctx.enter_context(tc.tile_pool(name="srpool", bufs=16))
        small = ctx.enter_context(tc.tile_pool(name="small", bufs=2))
        ppool = ctx.enter_context(tc.tile_pool(name="psum_main", bufs=2, space="PSUM"))
        ppre = ctx.enter_context(tc.tile_pool(name="psum_prep", bufs=1, space="PSUM"))

        # ---------------- persistent tiles ----------------
        cap_sb = const.tile([128, NCH, BC, T], BF16)
        img_sb = const.tile([128, NCH, ISH, R], F32)
        wg_sb = const.tile([128, NCH, D], BF16)
        wb_sb = const.tile([128, NCH, D], BF16)
        bg_sb = const.tile([128, NCH], F32)
        bb_sb = const.tile([128, NCH], F32)
        bg1_sb = const.tile([128, NCH], F32)
        ones_sb = const.tile([128, 1], F32)
        eps_sb = const.tile([128, 1], F32)
        id6 = const.tile([ISH, ISH], F32)
        xmm = const.tile([128, NCH, BC, 2], F32)
        q_sb = const.tile([128, NCH, ISH], F32)
        q_bf = const.tile([128, NCH, ISH], BF16)
        mv_all = const.tile([128, NCH, 2], F32)
        lnv = const.tile([128, NCH], F32)
        istd = const.tile([128, NCH], F32)
        a_sb = const.tile([128, NCH, ISH], F32)
        b_sb = const.tile([128, NCH, ISH], F32)
        sc_sb = const.tile([128, NCH, ISH], F32)
        bi_sb = const.tile([128, NCH, ISH], F32)
        tmp_sb = const.tile([128, NCH, ISH], F32)
        gfl = const.tile([ISH, D], F32)
        bfl = const.tile([ISH, D], F32)
        nq_s = const.tile([1, ISH], F32)
        rq_sb = const.tile([1, ISH], F32)
        dot_all = const.tile([1, ISH, BC], F32)
        ss_all = const.tile([1, ISH, BC], F32)
        lnss = const.tile([1, ISH, BC], F32)
        rsd = const.tile([1, ISH, BC], F32)
        prod_all = const.tile([1, ISH, BC], F32)
        out_sb = const.tile([1, ISH, BC], F32)

        capT_v = capT[:].rearrange("(k p) c t -> p k c t", p=128)
        imgT_v = imgT[:].rearrange("(k p) i r -> p k i r", p=128)
        wgT_v = wgT[:].rearrange("(k p) d -> p k d", p=128)
        wbT_v = wbT[:].rearrange("(k p) d -> p k d", p=128)

        nc.vector.memset(ones_sb[:], 1.0)
        nc.vector.memset(eps_sb[:], EPS)

        # ---------------- DMA in ----------------
        # HWDGE via the sync engine; Bacc's event-semaphore legalization
        # splits any multi-domain waits on consumers. Order by need: img/W
        # feed the gamma projections (longest prep chain), cap streams after.
        nc.sync.dma_start(out=img_sb[:], in_=imgT_v[:])
        nc.sync.dma_start(out=id6[:], in_=ident[:])
        nc.sync.dma_start(out=bg_sb[:], in_=bgp[:])
        nc.sync.dma_start(out=bb_sb[:], in_=bbp[:])
        for k in range(NCH):
            nc.sync.dma_start(out=cap_sb[:, k], in_=capT_v[:, k])
        for k in range(NCH):
            nc.sync.dma_start(out=wg_sb[:, k], in_=wgT_v[:, k])
            nc.sync.dma_start(out=wb_sb[:, k], in_=wbT_v[:, k])

        # ---------------- prep ----------------
        nc.vector.tensor_scalar_add(bg1_sb[:], bg_sb[:], 1.0)
        bb1_sb = const.tile([128, NCH], F32)
        nc.vector.tensor_copy(bb1_sb[:], bb_sb[:])

        # image query: sum over regions (mean folded later)
        nc.vector.reduce_sum(q_sb[:], img_sb[:], axis=AX.X)
        nc.vector.tensor_copy(q_bf[:], q_sb[:])

        # gamma/beta projections: psum (ISH, 512) accumulated over k-chunks
        for w_sb, flat in ((wg_sb, gfl), (wb_sb, bfl)):
            for h in range(2):
                ps = ppre.tile([ISH, 512], F32, tag="gb")
                for k in range(NCH):
                    nc.tensor.matmul(
                        ps[:], q_bf[:, k], w_sb[:, k, h * 512:(h + 1) * 512],
                        start=(k == 0), stop=(k == NCH - 1),
                    )
                nc.scalar.copy(flat[:, h * 512:(h + 1) * 512], ps[:])

        # per-chunk BN -> istd -> fold first: sc/bi[k] is all the y-exp stream
        # needs, so ACT unblocks as early as possible.
        for k in range(NCH):
            cap_flat = cap_sb[:, k].rearrange("p c t -> p (c t)")
            stats = small.tile([128, 4, 6], F32, tag="bnstats")
            for g in range(4):
                nc.vector.bn_stats(out=stats[:, g], in_=cap_flat[:, g * 480:(g + 1) * 480])
            nc.vector.bn_aggr(out=mv_all[:, k], in_=stats[:])

        # batched istd: one Ln + one Exp -> two ACT table loads total
        nc.scalar.activation(lnv[:], mv_all[:, :, 1], func=ACT.Ln, bias=eps_sb[:], scale=1.0)
        nc.scalar.activation(istd[:], lnv[:], func=ACT.Exp, scale=-0.5)

        for k in range(NCH):
            ptg = ppre.tile([128, ISH], F32, tag="tp")
            nc.tensor.transpose(ptg[:], gfl[:, k * 128:(k + 1) * 128], id6[:])
            nc.scalar.activation(a_sb[:, k], ptg[:], func=ACT.Identity,
                                 scale=1.0 / R, bias=bg1_sb[:, k:k + 1])
            ptb = ppre.tile([128, ISH], F32, tag="tp")
            nc.tensor.transpose(ptb[:], bfl[:, k * 128:(k + 1) * 128], id6[:])
            nc.scalar.activation(b_sb[:, k], ptb[:], func=ACT.Identity,
                                 scale=1.0 / R, bias=bb1_sb[:, k:k + 1])
            # sc = a * istd ; bi = b - sc * mean
            nc.vector.tensor_scalar(sc_sb[:, k], a_sb[:, k], istd[:, k:k + 1], None, op0=ALU.mult)
            nc.vector.tensor_scalar(tmp_sb[:, k], sc_sb[:, k], mv_all[:, k, 0:1], None, op0=ALU.mult)
            nc.vector.tensor_sub(bi_sb[:, k], b_sb[:, k], tmp_sb[:, k])

        # min/max over t per (c, d): 2x bf16 tree stages + final 1x reduce.
        # Off the y-exp critical path; only gates the first tv-multiplies.
        for k in range(NCH):
            xt1 = small.tile([128, BC, TH], BF16, tag="xt1")
            xt2 = small.tile([128, BC, TQ], BF16, tag="xt2")
            nc.vector.tensor_tensor(xt1[:], cap_sb[:, k, :, 0:TH], cap_sb[:, k, :, TH:T], op=ALU.max)
            nc.vector.tensor_tensor(xt2[:], xt1[:, :, 0:TQ], xt1[:, :, TQ:TH], op=ALU.max)
            nc.vector.tensor_reduce(xmm[:, k, :, 1:2], xt2[:], axis=AX.X, op=ALU.max)
            xn1 = small.tile([128, BC, TH], BF16, tag="xn1")
            xn2 = small.tile([128, BC, TQ], BF16, tag="xn2")
            nc.vector.tensor_tensor(xn1[:], cap_sb[:, k, :, 0:TH], cap_sb[:, k, :, TH:T], op=ALU.min)
            nc.vector.tensor_tensor(xn2[:], xn1[:, :, 0:TQ], xn1[:, :, TQ:TH], op=ALU.min)
            nc.vector.tensor_reduce(xmm[:, k, :, 0:1], xn2[:], axis=AX.X, op=ALU.min)

        # ||q_sum||^-1 per image, landing on partition 0 as (1, ISH)
        nqp = ppre.tile([1, ISH], F32, tag="nq")
        sq_all = const.tile([128, NCH, ISH], F32)
        nc.vector.tensor_mul(sq_all[:], q_sb[:], q_sb[:])
        for k in range(NCH):
            nc.tensor.matmul(nqp[:], ones_sb[:], sq_all[:, k], start=(k == 0), stop=(k == NCH - 1))
        nc.scalar.activation(nq_s[:], nqp[:], func=ACT.Ln, bias=0.0, scale=1.0)
        nc.scalar.activation(rq_sb[:], nq_s[:], func=ACT.Exp, scale=-0.5)

        # ---------------- main loop ----------------
        for i in range(ISH):
            c12 = ipool.tile([128, NCH, BC, 2], F32, tag="c12")
            e12 = ipool.tile([128, NCH, BC, 2], F32, tag="e12")
            f12 = ipool.tile([128, NCH, BC, 2], F32, tag="f12")
            m_all = ipool.tile([128, NCH, BC], F32, tag="m")
            for k in range(NCH):
                nc.vector.tensor_scalar(
                    c12[:, k], xmm[:, k], sc_sb[:, k, i:i + 1], bi_sb[:, k, i:i + 1],
                    op0=ALU.mult, op1=ALU.add,
                )
            nc.scalar.activation(e12[:], c12[:], func=ACT.Exp)
            nc.vector.tensor_mul(f12[:], c12[:], e12[:])
            nc.vector.tensor_tensor(m_all[:], f12[:, :, :, 0], f12[:, :, :, 1], op=ALU.max)

            pd = ppool.tile([1, 2 * BC], F32, tag="pdot")
            pss = ppool.tile([1, 2 * BC], F32, tag="pss")
            for k in range(NCH):
                y = ypool.tile([128, BC, T], BF16, tag="y")
                nc.scalar.activation(y[:], cap_sb[:, k], func=ACT.Exp,
                                     scale=sc_sb[:, k, i:i + 1], bias=bi_sb[:, k, i:i + 1])
                ys1 = work.tile([128, BC, TH], BF16, tag="ys1")
                nc.vector.tensor_add(ys1[:], y[:, :, 0:TH], y[:, :, TH:T])
                ys2 = work.tile([128, BC, TQ], BF16, tag="ys2")
                nc.vector.tensor_add(ys2[:], ys1[:, :, 0:TQ], ys1[:, :, TQ:TH])
                s_t = srpool.tile([128, BC], F32, tag="s")
                nc.vector.reduce_sum(s_t[:], ys2[:], axis=AX.X)
                r_t = srpool.tile([128, BC], F32, tag="r")
                nc.vector.reciprocal_approx_fast(r_t[:], s_t[:])
                tv = work.tile([128, 2, BC], F32, tag="tv")
                nc.vector.tensor_mul(tv[:, 0], m_all[:, k], r_t[:])
                nc.vector.tensor_mul(tv[:, 1], tv[:, 0], tv[:, 0])
                nc.tensor.matmul(pd[:], q_sb[:, k, i:i + 1], tv[:],
                                 start=(k == 0), stop=(k == NCH - 1))
                nc.tensor.matmul(pss[:], ones_sb[:], tv[:],
                                 start=(k == 0), stop=(k == NCH - 1))
            nc.vector.tensor_copy(dot_all[:, i], pd[0:1, 0:BC])
            nc.vector.tensor_copy(ss_all[:, i], pss[0:1, BC:2 * BC])

        # ---------------- tail ----------------
        nc.scalar.activation(lnss[:], ss_all[:], func=ACT.Ln, bias=0.0, scale=1.0)
        nc.scalar.activation(rsd[:], lnss[:], func=ACT.Exp, scale=-0.5)
        nc.vector.tensor_mul(prod_all[:], dot_all[:], rsd[:])
        for i in range(ISH):
            nc.vector.tensor_scalar(out_sb[0:1, i], prod_all[0:1, i],
                                    rq_sb[0:1, i:i + 1], None, op0=ALU.mult)
        nc.sync.dma_start(out=out_e[:].rearrange("i c -> (i c)"),
                          in_=out_sb[:].rearrange("p i c -> p (i c)"))

    nc.compile()
    return nc


_NC = None
LAST_RESULT = None


def _get_nc():
    global _NC
    if _NC is None:
        _NC = build_bass()
    return _NC


def kernel(img_embed, cap_embed, lens=None, W_gamma=None, b_gamma=None,
           W_beta=None, b_beta=None, **_unused):
    global LAST_RESULT
    img = np.asarray(img_embed, dtype=np.float32)
    cap = np.asarray(cap_embed, dtype=np.float32)
    Wg = np.asarray(W_gamma, dtype=np.float32)
    Wb = np.asarray(W_beta, dtype=np.float32)
    bg = np.asarray(b_gamma, dtype=np.float32)
    bb = np.asarray(b_beta, dtype=np.float32)

    capT = np.ascontiguousarray(cap.transpose(2, 0, 1)).astype(ml_dtypes.bfloat16)
    wgT = np.ascontiguousarray(Wg.T).astype(ml_dtypes.bfloat16)
    wbT = np.ascontiguousarray(Wb.T).astype(ml_dtypes.bfloat16)
    bgp = np.ascontiguousarray(bg.reshape(NCH, 128).T).astype(np.float32)
    bbp = np.ascontiguousarray(bb.reshape(NCH, 128).T).astype(np.float32)

    in_maps = []
    for c in range(NCORES):
        i0 = c * ISH
        imgT_c = np.ascontiguousarray(
            img[i0:i0 + ISH].transpose(2, 0, 1)).astype(np.float32)
        in_maps.append({
            "capT": capT, "imgT": imgT_c, "wgT": wgT, "wbT": wbT,
            "bgp": bgp, "bbp": bbp, "ident": np.eye(ISH, dtype=np.float32),
        })

    nc = _get_nc()
    res = run_bass_kernel_spmd(nc, in_maps, core_ids=list(range(NCORES)))
    LAST_RESULT = res
    out = np.concatenate(
        [np.asarray(res.results[c]["out"], dtype=np.float32) for c in range(NCORES)],
        axis=0,
    )
    return out


if __name__ == "__main__":
    rng = np.random.default_rng(0)
    ins = dict(
        img_embed=rng.standard_normal((BI, R, D), dtype=np.float32),
        cap_embed=rng.standard_normal((BC, T, D), dtype=np.float32),
        lens=rng.integers(1, T, size=(BC,)),
        W_gamma=(rng.standard_normal((D, D), dtype=np.float32) / np.sqrt(D)).astype(np.float32),
        b_gamma=np.zeros((D,), np.float32),
        W_beta=(rng.standard_normal((D, D), dtype=np.float32) / np.sqrt(D)).astype(np.float32),
        b_beta=np.zeros((D,), np.float32),
    )
    o = kernel(**ins)
    print(o.shape, o.dtype, float(np.abs(o).mean()))


# revision 27
# speedup vs baseline: 1.2868x; 1.0171x over previous
"""Trainium2 Bass kernel for nn_AdaptiveEmbeddingI2T (retrieval_knn).

Data-parallel over the image axis: 8 cores, 6 images each; cap_embed and
the gamma/beta projection weights are replicated. Each core computes its
(6, 48) slab of the similarity matrix; the host concatenates.

Math notes (per image i, caption c, channel d):
  txt[c,d,t] = sc[i,d] * cap[d,c,t] + bi[i,d]      (BN + FiLM folded into one affine)
  mask = softmax_t(txt);  tv[c,d] = max_t(mask*txt) = max_t(f(txt_t)) / sum_t exp(txt_t)
  f(x) = x*exp(x) is decreasing-then-increasing, so max_t f(txt_t) is attained at
  an endpoint of txt's range; txt is affine in cap, so the endpoints come from
  min_t/max_t of cap -- computed ONCE (image-independent).
  sims[i,c] = (q_sum . tv) / (||q_sum|| * ||tv||)   (the 1/R mean factor cancels)
"""

import os
import sys

import numpy as np


def _ensure_import():
    try:
        import concourse.bass  # noqa: F401
        return
    except Exception:
        pass
    for p in ("/opt/trn_rl_repo", "/root/.axon_site/_ro/trn_rl_repo"):
        if os.path.isdir(p) and p not in sys.path:
            sys.path.insert(0, p)
    import concourse.bass  # noqa: F401


_ensure_import()


def _install_axon_profile_shim():
    """The image's antenv lacks axon_hooks; synthesize it so trace=True under
    axon can register the ctypes NTFF profiling hook from trn_boot."""
    try:
        import antenv.axon_hooks  # noqa: F401
        return
    except Exception:
        pass
    try:
        import types

        import antenv

        mod = types.ModuleType("antenv.axon_hooks")
        holder = {"h": None}
        mod.set_axon_ntff_profile_hook = lambda h: holder.__setitem__("h", h)
        mod.get_axon_ntff_profile_hook = lambda: holder["h"]
        sys.modules["antenv.axon_hooks"] = mod
        antenv.axon_hooks = mod

        boot_dir = "/root/.axon_site/trn_agent_boot"
        so_path = "/opt/axon/libaxon_pjrt.so"
        if os.path.isdir(boot_dir) and os.path.exists(so_path):
            if boot_dir not in sys.path:
                sys.path.insert(0, boot_dir)
            import trn_boot

            h = trn_boot._ntff_profile_via_ctypes(so_path)
            if h is not None:
                mod.set_axon_ntff_profile_hook(h)
    except Exception:
        pass


_install_axon_profile_shim()

from contextlib import ExitStack  # noqa: E402

import ml_dtypes  # noqa: E402

import concourse.bass as bass  # noqa: E402
import concourse.bacc as bacc  # noqa: E402
import concourse.tile as tile  # noqa: E402
from concourse import mybir  # noqa: E402
from concourse.bass_utils import run_bass_kernel_spmd  # noqa: E402

F32 = mybir.dt.float32
BF16 = mybir.dt.bfloat16
AX = mybir.AxisListType
ALU = mybir.AluOpType
ACT = mybir.ActivationFunctionType

D, BI, BC, R, T = 1024, 48, 48, 36, 40
NCORES = 8
ISH = BI // NCORES  # 6 images per core
NCH = D // 128      # 8 d-chunks
EPS = 1e-5
TH = T // 2         # 20
TQ = T // 4         # 10


def build_bass():
    nc = bacc.Bacc("TRN2", target_bir_lowering=False)
    capT = nc.declare_dram_parameter("capT", [D, BC, T], BF16, isOutput=False)
    imgT = nc.declare_dram_parameter("imgT", [D, ISH, R], F32, isOutput=False)
    wgT = nc.declare_dram_parameter("wgT", [D, D], BF16, isOutput=False)
    wbT = nc.declare_dram_parameter("wbT", [D, D], BF16, isOutput=False)
    bgp = nc.declare_dram_parameter("bgp", [128, NCH], F32, isOutput=False)
    bbp = nc.declare_dram_parameter("bbp", [128, NCH], F32, isOutput=False)
    ident = nc.declare_dram_parameter("ident", [ISH, ISH], F32, isOutput=False)
    out_e = nc.declare_dram_parameter("out", [ISH, BC], F32, isOutput=True)

    with ExitStack() as ctx:
        tc = ctx.enter_context(tile.TileContext(nc))
        const = ctx.enter_context(tc.tile_pool(name="const", bufs=1))
        work = ctx.enter_context(tc.tile_pool(name="work", bufs=4))
        ypool = ctx.enter_context(tc.tile_pool(name="ypool", bufs=8))
        ipool = ctx.enter_context(tc.tile_pool(name="ipool", bufs=2))
        srpool = ctx.enter_context(tc.tile_pool(name="srpool", bufs=16))
        small = ctx.enter_context(tc.tile_pool(name="small", bufs=2))
        ppool = ctx.enter_context(tc.tile_pool(name="psum_main", bufs=2, space="PSUM"))
        ppre = ctx.enter_context(tc.tile_pool(name="psum_prep", bufs=1, space="PSUM"))

        # ---------------- persistent tiles ----------------
        cap_sb = const.tile([128, NCH, BC, T], BF16)
        img_sb = const.tile([128, NCH, ISH, R], F32)
        wg_sb = const.tile([128, NCH, D], BF16)
        wb_sb = const.tile([128, NCH, D], BF16)
        bg_sb = const.tile([128, NCH], F32)
        bb_sb = const.tile([128, NCH], F32)
        bg1_sb = const.tile([128, NCH], F32)
        ones_sb = const.tile([128, 1], F32)
        eps_sb = const.tile([128, 1], F32)
        id6 = const.tile([ISH, ISH], F32)
        xmm = const.tile([128, NCH, BC, 2], BF16)
        q_sb = const.tile([128, NCH, ISH], F32)
        q_bf = const.tile([128, NCH, ISH], BF16)
        mv_all = const.tile([128, NCH, 2], F32)
        lnv = const.tile([128, NCH], F32)
        istd = const.tile([128, NCH], F32)
        a_sb = const.tile([128, NCH, ISH], F32)
        b_sb = const.tile([128, NCH, ISH], F32)
        sc_sb = const.tile([128, NCH, ISH], F32)
        bi_sb = const.tile([128, NCH, ISH], F32)
        tmp_sb = const.tile([128, NCH, ISH], F32)
        gfl = const.tile([ISH, D], F32)
        bfl = const.tile([ISH, D], F32)
        nq_s = const.tile([1, ISH], F32)
        rq_sb = const.tile([1, ISH], F32)
        dot_all = const.tile([1, ISH, BC], F32)
        ss_all = const.tile([1, ISH, BC], F32)
        lnss = const.tile([1, ISH, BC], F32)
        rsd = const.tile([1, ISH, BC], F32)
        prod_all = const.tile([1, ISH, BC], F32)
        out_sb = const.tile([1, ISH, BC], F32)

        capT_v = capT[:].rearrange("(k p) c t -> p k c t", p=128)
        imgT_v = imgT[:].rearrange("(k p) i r -> p k i r", p=128)
        wgT_v = wgT[:].rearrange("(k p) d -> p k d", p=128)
        wbT_v = wbT[:].rearrange("(k p) d -> p k d", p=128)

        nc.vector.memset(ones_sb[:], 1.0)
        nc.vector.memset(eps_sb[:], EPS)

        # ---------------- DMA in ----------------
        # HWDGE via the sync engine; Bacc's event-semaphore legalization
        # splits any multi-domain waits on consumers. Order by need: img/W
        # feed the gamma projections (longest prep chain), cap streams after.
        nc.sync.dma_start(out=img_sb[:], in_=imgT_v[:])
        nc.sync.dma_start(out=id6[:], in_=ident[:])
        nc.sync.dma_start(out=bg_sb[:], in_=bgp[:])
        nc.sync.dma_start(out=bb_sb[:], in_=bbp[:])
        for k in range(NCH):
            nc.sync.dma_start(out=cap_sb[:, k], in_=capT_v[:, k])
        for k in range(NCH):
            nc.sync.dma_start(out=wg_sb[:, k], in_=wgT_v[:, k])
            nc.sync.dma_start(out=wb_sb[:, k], in_=wbT_v[:, k])

        # ---------------- prep ----------------
        nc.vector.tensor_scalar_add(bg1_sb[:], bg_sb[:], 1.0)
        bb1_sb = const.tile([128, NCH], F32)
        nc.vector.tensor_copy(bb1_sb[:], bb_sb[:])

        # image query: sum over regions (mean folded later)
        nc.vector.reduce_sum(q_sb[:], img_sb[:], axis=AX.X)
        nc.vector.tensor_copy(q_bf[:], q_sb[:])

        # gamma/beta projections: psum (ISH, 512) accumulated over k-chunks
        for w_sb, flat in ((wg_sb, gfl), (wb_sb, bfl)):
            for h in range(2):
                ps = ppre.tile([ISH, 512], F32, tag="gb")
                for k in range(NCH):
                    nc.tensor.matmul(
                        ps[:], q_bf[:, k], w_sb[:, k, h * 512:(h + 1) * 512],
                        start=(k == 0), stop=(k == NCH - 1),
                    )
                nc.scalar.copy(flat[:, h * 512:(h + 1) * 512], ps[:])

        # per-chunk BN -> istd -> fold first: sc/bi[k] is all the y-exp stream
        # needs, so ACT unblocks as early as possible.
        for k in range(NCH):
            cap_flat = cap_sb[:, k].rearrange("p c t -> p (c t)")
            stats = small.tile([128, 4, 6], F32, tag="bnstats")
            for g in range(4):
                nc.vector.bn_stats(out=stats[:, g], in_=cap_flat[:, g * 480:(g + 1) * 480])
            nc.vector.bn_aggr(out=mv_all[:, k], in_=stats[:])

        # batched istd: one Ln + one Exp -> two ACT table loads total
        nc.scalar.activation(lnv[:], mv_all[:, :, 1], func=ACT.Ln, bias=eps_sb[:], scale=1.0)
        nc.scalar.activation(istd[:], lnv[:], func=ACT.Exp, scale=-0.5)

        for k in range(NCH):
            ptg = ppre.tile([128, ISH], F32, tag="tp")
            nc.tensor.transpose(ptg[:], gfl[:, k * 128:(k + 1) * 128], id6[:])
            nc.scalar.activation(a_sb[:, k], ptg[:], func=ACT.Identity,
                                 scale=1.0 / R, bias=bg1_sb[:, k:k + 1])
            ptb = ppre.tile([128, ISH], F32, tag="tp")
            nc.tensor.transpose(ptb[:], bfl[:, k * 128:(k + 1) * 128], id6[:])
            nc.scalar.activation(b_sb[:, k], ptb[:], func=ACT.Identity,
                                 scale=1.0 / R, bias=bb1_sb[:, k:k + 1])
            # sc = a * istd ; bi = b - sc * mean
            nc.vector.tensor_scalar(sc_sb[:, k], a_sb[:, k], istd[:, k:k + 1], None, op0=ALU.mult)
            nc.vector.tensor_scalar(tmp_sb[:, k], sc_sb[:, k], mv_all[:, k, 0:1], None, op0=ALU.mult)
            nc.vector.tensor_sub(bi_sb[:, k], b_sb[:, k], tmp_sb[:, k])

        # min/max over t per (c, d): 2x bf16 tree stages + final 1x reduce.
        # Off the y-exp critical path; only gates the first tv-multiplies.
        for k in range(NCH):
            xt1 = small.tile([128, BC, TH], BF16, tag="xt1")
            xt2 = small.tile([128, BC, TQ], BF16, tag="xt2")
            nc.vector.tensor_tensor(xt1[:], cap_sb[:, k, :, 0:TH], cap_sb[:, k, :, TH:T], op=ALU.max)
            nc.vector.tensor_tensor(xt2[:], xt1[:, :, 0:TQ], xt1[:, :, TQ:TH], op=ALU.max)
            nc.vector.tensor_reduce(xmm[:, k, :, 1:2], xt2[:], axis=AX.X, op=ALU.max)
            xn1 = small.tile([128, BC, TH], BF16, tag="xn1")
            xn2 = small.tile([128, BC, TQ], BF16, tag="xn2")
            nc.vector.tensor_tensor(xn1[:], cap_sb[:, k, :, 0:TH], cap_sb[:, k, :, TH:T], op=ALU.min)
            nc.vector.tensor_tensor(xn2[:], xn1[:, :, 0:TQ], xn1[:, :, TQ:TH], op=ALU.min)
            nc.vector.tensor_reduce(xmm[:, k, :, 0:1], xn2[:], axis=AX.X, op=ALU.min)

        # ||q_sum||^-1 per image, landing on partition 0 as (1, ISH)
        nqp = ppre.tile([1, ISH], F32, tag="nq")
        sq_all = const.tile([128, NCH, ISH], F32)
        nc.vector.tensor_mul(sq_all[:], q_sb[:], q_sb[:])
        for k in range(NCH):
            nc.tensor.matmul(nqp[:], ones_sb[:], sq_all[:, k], start=(k == 0), stop=(k == NCH - 1))
        nc.scalar.activation(nq_s[:], nqp[:], func=ACT.Ln, bias=0.0, scale=1.0)
        nc.scalar.activation(rq_sb[:], nq_s[:], func=ACT.Exp, scale=-0.5)

        # ---------------- main loop ----------------
        for i in range(ISH):
            c12 = ipool.tile([128, NCH, BC, 2], BF16, tag="c12")
            e12 = ipool.tile([128, NCH, BC, 2], BF16, tag="e12")
            f12 = ipool.tile([128, NCH, BC, 2], BF16, tag="f12")
            m_all = ipool.tile([128, NCH, BC], F32, tag="m")
            for k in range(NCH):
                nc.vector.tensor_scalar(
                    c12[:, k], xmm[:, k], sc_sb[:, k, i:i + 1], bi_sb[:, k, i:i + 1],
                    op0=ALU.mult, op1=ALU.add,
                )
            nc.scalar.activation(e12[:], c12[:], func=ACT.Exp)
            nc.vector.tensor_mul(f12[:], c12[:], e12[:])
            nc.vector.tensor_tensor(m_all[:], f12[:, :, :, 0], f12[:, :, :, 1], op=ALU.max)

            pd = ppool.tile([1, 2 * BC], F32, tag="pdot")
            pss = ppool.tile([1, 2 * BC], F32, tag="pss")
            for k in range(NCH):
                y = ypool.tile([128, BC, T], BF16, tag="y")
                nc.scalar.activation(y[:], cap_sb[:, k], func=ACT.Exp,
                                     scale=sc_sb[:, k, i:i + 1], bias=bi_sb[:, k, i:i + 1])
                ys1 = work.tile([128, BC, TH], BF16, tag="ys1")
                nc.vector.tensor_add(ys1[:], y[:, :, 0:TH], y[:, :, TH:T])
                ys2 = work.tile([128, BC, TQ], BF16, tag="ys2")
                nc.vector.tensor_add(ys2[:], ys1[:, :, 0:TQ], ys1[:, :, TQ:TH])
                s_t = srpool.tile([128, BC], F32, tag="s")
                nc.vector.reduce_sum(s_t[:], ys2[:], axis=AX.X)
                r_t = srpool.tile([128, BC], F32, tag="r")
                nc.vector.reciprocal_approx_fast(r_t[:], s_t[:])
                tv = work.tile([128, 2, BC], F32, tag="tv")
                nc.vector.tensor_mul(tv[:, 0], m_all[:, k], r_t[:])
                nc.scalar.square(tv[:, 1], tv[:, 0])
                nc.tensor.matmul(pd[:], q_sb[:, k, i:i + 1], tv[:],
                                 start=(k == 0), stop=(k == NCH - 1))
                nc.tensor.matmul(pss[:], ones_sb[:], tv[:],
                                 start=(k == 0), stop=(k == NCH - 1))
            nc.vector.tensor_copy(dot_all[:, i], pd[0:1, 0:BC])
            nc.vector.tensor_copy(ss_all[:, i], pss[0:1, BC:2 * BC])

        # ---------------- tail ----------------
        nc.scalar.activation(lnss[:], ss_all[:], func=ACT.Ln, bias=0.0, scale=1.0)
        nc.scalar.activation(rsd[:], lnss[:], func=ACT.Exp, scale=-0.5)
        nc.vector.tensor_mul(prod_all[:], dot_all[:], rsd[:])
        for i in range(ISH):
            nc.vector.tensor_scalar(out_sb[0:1, i], prod_all[0:1, i],
                                    rq_sb[0:1, i:i + 1], None, op0=ALU.mult)
        nc.sync.dma_start(out=out_e[:].rearrange("i c -> (i c)"),
                          in_=out_sb[:].rearrange("p i c -> p (i c)"))

    nc.compile()
    return nc


_NC = None
LAST_RESULT = None


def _get_nc():
    global _NC
    if _NC is None:
        _NC = build_bass()
    return _NC


def kernel(img_embed, cap_embed, lens=None, W_gamma=None, b_gamma=None,
           W_beta=None, b_beta=None, **_unused):
    global LAST_RESULT
    img = np.asarray(img_embed, dtype=np.float32)
    cap = np.asarray(cap_embed, dtype=np.float32)
    Wg = np.asarray(W_gamma, dtype=np.float32)
    Wb = np.asarray(W_beta, dtype=np.float32)
    bg = np.asarray(b_gamma, dtype=np.float32)
    bb = np.asarray(b_beta, dtype=np.float32)

    capT = np.ascontiguousarray(cap.transpose(2, 0, 1)).astype(ml_dtypes.bfloat16)
    wgT = np.ascontiguousarray(Wg.T).astype(ml_dtypes.bfloat16)
    wbT = np.ascontiguousarray(Wb.T).astype(ml_dtypes.bfloat16)
    bgp = np.ascontiguousarray(bg.reshape(NCH, 128).T).astype(np.float32)
    bbp = np.ascontiguousarray(bb.reshape(NCH, 128).T).astype(np.float32)

    in_maps = []
    for c in range(NCORES):
        i0 = c * ISH
        imgT_c = np.ascontiguousarray(
            img[i0:i0 + ISH].transpose(2, 0, 1)).astype(np.float32)
        in_maps.append({
            "capT": capT, "imgT": imgT_c, "wgT": wgT, "wbT": wbT,
            "bgp": bgp, "bbp": bbp, "ident": np.eye(ISH, dtype=np.float32),
        })

    nc = _get_nc()
    res = run_bass_kernel_spmd(nc, in_maps, core_ids=list(range(NCORES)))
    LAST_RESULT = res
    out = np.concatenate(
        [np.asarray(res.results[c]["out"], dtype=np.float32) for c in range(NCORES)],
        axis=0,
    )
    return out


if __name__ == "__main__":
    rng = np.random.default_rng(0)
    ins = dict(
        img_embed=rng.standard_normal((BI, R, D), dtype=np.float32),
        cap_embed=rng.standard_normal((BC, T, D), dtype=np.float32),
        lens=rng.integers(1, T, size=(BC,)),
        W_gamma=(rng.standard_normal((D, D), dtype=np.float32) / np.sqrt(D)).astype(np.float32),
        b_gamma=np.zeros((D,), np.float32),
        W_beta=(rng.standard_normal((D, D), dtype=np.float32) / np.sqrt(D)).astype(np.float32),
        b_beta=np.zeros((D,), np.float32),
    )
    o = kernel(**ins)
    print(o.shape, o.dtype, float(np.abs(o).mean()))
